# revision 16
# baseline (speedup 1.0000x reference)
"""Distributed MemoryCenters read kernel for 8 Trainium2 NeuronCores.

Strategy (sharded-kNN per the distributed top-k pattern):
  - Shard the center table K along n_centers across the 8 cores
    (12500 centers each). Queries are replicated.
  - Each core computes sim = q @ K_shard^T on the PE. To get fp32-grade
    precision at full PE rate, operands are split hi/lo (hi = fp16 with
    subnormals flushed on host, lo = bf16 residual) and accumulated as
    qh*Kh + qh*Kl + ql*Kh in one PSUM group (measured max err 1.9e-7,
    same as fp32).
  - Each core extracts top-8 candidates (values + in-chunk indices) per
    2048-wide chunk of its shard with the DVE max8 / find_index8 ops.
  - The host merges the 8 * 56 = 448 candidates per query, takes the
    global top-32 by RBF weight (reproducing the reference's ordering and
    tie-breaking), and performs the cheap O(k) softmax / gather reduction.

Exactness: top-8 per 2048-chunk covers the global top-32 as long as no
chunk holds more than 8 of the top-32 (actual maximum on this data is 5;
test.py's saturation check proves this per-run).
"""

import numpy as np

SIGMA_READ = 0.5
EPS = 1e-8

B, T, D = 2, 512, 128
N, DV, DE = 100000, 256, 4
NCORES = 8
NSHARD = N // NCORES  # 12500
Q = B * T  # 1024
QTILES = Q // 128  # 8

CHUNK = 2048
# chunk widths covering the 12500-wide shard; small leading chunks let the
# DVE start as soon as the first matmul pieces land (pipeline fill)
_WIDTHS = [512, 512, 1024, 2048, 2048, 2048, 2048, 2260]
assert sum(_WIDTHS) == NSHARD
CHUNKS = []
_off = 0
for _w in _WIDTHS:
    CHUNKS.append((_off, _w))
    _off += _w
NCHUNK = len(CHUNKS)  # 9
CAND = NCHUNK * 8  # 72 candidates per (query, core)

F16_MIN_NORMAL = 6.2e-05  # flush-to-zero threshold for the hi fp16 part

_CACHE = {}


def _build_bass():
    """Build + compile the per-core Bass program (identical on all cores)."""
    from contextlib import ExitStack

    import concourse.bacc as bacc
    import concourse.mybir as mybir
    import concourse.tile as tile

    f32 = mybir.dt.float32
    f16 = mybir.dt.float16
    bf16 = mybir.dt.bfloat16
    u32 = mybir.dt.uint32

    nc = bacc.Bacc("TRN2", target_bir_lowering=False, debug=False,
                   enable_asserts=False, num_devices=NCORES)

    qh = nc.dram_tensor("qh", [128, Q], f16, kind="ExternalInput").ap()
    ql = nc.dram_tensor("ql", [128, Q], bf16, kind="ExternalInput").ap()
    kh = nc.dram_tensor("kh", [128, NSHARD], f16, kind="ExternalInput").ap()
    kl = nc.dram_tensor("kl", [128, NSHARD], bf16, kind="ExternalInput").ap()
    outv = nc.dram_tensor("outv", [QTILES, 128, CAND], f32, kind="ExternalOutput").ap()
    outi = nc.dram_tensor("outi", [QTILES, 128, CAND], u32, kind="ExternalOutput").ap()

    with tile.TileContext(nc) as tc, ExitStack() as ctx:
        k_pool = ctx.enter_context(tc.tile_pool(name="k", bufs=1))
        q_pool = ctx.enter_context(tc.tile_pool(name="q", bufs=1))
        # 2-bank PSUM pieces, 4 in flight: lets the PE run ahead of the
        # ACT drain so it can ramp to its top p-state
        psum_pool = ctx.enter_context(tc.tile_pool(name="ps", bufs=4, space="PSUM"))
        # deep SBUF runway so ACT (and transitively PE) is not paced by DVE
        chunk_pool = ctx.enter_context(tc.tile_pool(name="ch", bufs=7))
        out_pool = ctx.enter_context(tc.tile_pool(name="out", bufs=2))

        # queries first (the PE's ldweights gate on them), on both HWDGE rings
        qh_sb = q_pool.tile([128, Q], f16, tag="qh")
        ql_sb = q_pool.tile([128, Q], bf16, tag="ql")
        nc.sync.dma_start(out=qh_sb[:], in_=qh[:])
        nc.scalar.dma_start(out=ql_sb[:], in_=ql[:])

        # K in chunk-aligned pieces as separate tiles so early matmuls only
        # gate on the piece they read; kh on the SP ring, kl on the ACT ring
        piece_bounds = [0, 512, 2048, 6144, 10240, NSHARD]
        pieces = [(piece_bounds[i], piece_bounds[i + 1] - piece_bounds[i])
                  for i in range(len(piece_bounds) - 1)]
        kh_sbs, kl_sbs = [], []
        for pi, (s, w) in enumerate(pieces):
            kh_t = k_pool.tile([128, w], f16, tag=f"kh{pi}")
            kl_t = k_pool.tile([128, w], bf16, tag=f"kl{pi}")
            nc.sync.dma_start(out=kh_t[:], in_=kh[:, s:s + w])
            nc.scalar.dma_start(out=kl_t[:], in_=kl[:, s:s + w])
            kh_sbs.append(kh_t)
            kl_sbs.append(kl_t)

        def k_slice(tiles, off, width):
            for (s, w), t in zip(pieces, tiles):
                if s <= off and off + width <= s + w:
                    return t[:, off - s:off - s + width]
            raise AssertionError((off, width))

        for t in range(QTILES):
            ov = out_pool.tile([128, CAND], f32, tag="ov")
            oi = out_pool.tile([128, CAND], u32, tag="oi")
            lh = qh_sb[:, t * 128:(t + 1) * 128]
            ll = ql_sb[:, t * 128:(t + 1) * 128]
            for ci, (off, w) in enumerate(CHUNKS):
                sb = chunk_pool.tile([128, max(_WIDTHS)], f32, tag="ch")
                # PSUM pieces of 1024 (2 banks); ACT drains each piece into
                # its slice of the chunk's SBUF buffer
                for p0 in range(0, w, 1024):
                    pw = min(1024, w - p0)
                    ps = psum_pool.tile([128, 1024], f32, tag="ps")
                    for j in range(0, pw, 512):
                        ww = min(512, pw - j)
                        rh = k_slice(kh_sbs, off + p0 + j, ww)
                        rl = k_slice(kl_sbs, off + p0 + j, ww)
                        po = ps[:, j:j + ww]
                        nc.tensor.matmul(po, lhsT=lh, rhs=rh, start=True, stop=False)
                        nc.tensor.matmul(po, lhsT=lh, rhs=rl, start=False, stop=False)
                        nc.tensor.matmul(po, lhsT=ll, rhs=rh, start=False, stop=True)
                    nc.scalar.activation(sb[:, p0:p0 + pw], ps[:, :pw],
                                         mybir.ActivationFunctionType.Copy)
                nc.vector.max(ov[:, ci * 8:(ci + 1) * 8], sb[:, :w])
                nc.vector.max_index(oi[:, ci * 8:(ci + 1) * 8],
                                    ov[:, ci * 8:(ci + 1) * 8], sb[:, :w])
            nc.sync.dma_start(out=outv[t], in_=ov[:])
            nc.sync.dma_start(out=outi[t], in_=oi[:])

    nc.compile()
    return nc


def _get_compiled():
    if "nc" not in _CACHE:
        _CACHE["nc"] = _build_bass()
    return _CACHE["nc"]


def _split_hi_lo(x):
    """x (f32) -> (hi fp16 with subnormals flushed, lo bf16), hi+lo ~ x."""
    import ml_dtypes
    xh = x.astype(np.float16)
    xh = np.where(np.abs(x) < F16_MIN_NORMAL, np.float16(0), xh)
    xl = (x - xh.astype(np.float32)).astype(ml_dtypes.bfloat16)
    return xh, xl


def build_in_maps(queries, K):
    qT = np.ascontiguousarray(queries.reshape(Q, D).T)  # [128, 1024]
    qh, ql = _split_hi_lo(qT)
    in_maps = []
    for c in range(NCORES):
        sh = np.ascontiguousarray(K[c * NSHARD:(c + 1) * NSHARD].T)  # [128, 12500]
        kh, kl = _split_hi_lo(sh)
        in_maps.append({"qh": qh, "ql": ql, "kh": kh, "kl": kl})
    return in_maps


def kernel(queries, K, V, h, e, top_k):
    top_k = int(top_k)
    assert top_k == 32, top_k
    queries = np.asarray(queries, dtype=np.float32)
    K = np.asarray(K, dtype=np.float32)
    V = np.asarray(V, dtype=np.float32)
    h = np.asarray(h, dtype=np.float32)
    e = np.asarray(e, dtype=np.float32)

    # ---- shard + run on 8 NeuronCores ----
    in_maps = build_in_maps(queries, K)

    from concourse.bass_utils import run_bass_kernel_spmd

    nc = _get_compiled()
    widths = np.repeat(np.array([w for _, w in CHUNKS], dtype=np.int64), 8)

    def _outputs_sane(res):
        # sims of unit vectors lie in [-1, 1]; indices are in-chunk offsets
        for c in range(NCORES):
            v = res[c]["outv"]
            if not np.isfinite(v).all() or np.abs(v).max() > 1.001:
                return False
            if (res[c]["outi"].astype(np.int64) >= widths[None, None, :]).any():
                return False
        return True

    res = run_bass_kernel_spmd(nc, in_maps, list(range(NCORES))).results
    if not _outputs_sane(res):  # transient device glitch: retry once
        res = run_bass_kernel_spmd(nc, in_maps, list(range(NCORES))).results

    # ---- unshard: merge the 8*56 candidates per query ----
    vals = np.stack([res[c]["outv"] for c in range(NCORES)])  # [8, QTILES, 128, CAND]
    idxs = np.stack([res[c]["outi"] for c in range(NCORES)]).astype(np.int64)
    cand_off = np.repeat(np.array([off for off, _ in CHUNKS], dtype=np.int64), 8)
    gidx = idxs + cand_off[None, None, None, :]
    gidx += (np.arange(NCORES, dtype=np.int64) * NSHARD)[:, None, None, None]

    v = vals.transpose(1, 2, 0, 3).reshape(Q, NCORES * CAND)
    gi = gidx.transpose(1, 2, 0, 3).reshape(Q, NCORES * CAND)

    # RBF weights, computed exactly as the reference does (f32 throughout)
    dist_sq = np.float32(2.0) - np.float32(2.0) * v
    rbf = np.exp(-dist_sq / np.float32(2.0 * SIGMA_READ ** 2)).astype(np.float32)

    # global top-32 by rbf, ties broken by lower center index (lax.top_k order)
    order = np.lexsort((gi, -rbf.astype(np.float64)), axis=1)[:, :top_k]
    topk_idx = np.take_along_axis(gi, order, axis=1)  # [Q, 32]
    topk_w = np.take_along_axis(rbf, order, axis=1)  # [Q, 32]

    # ---- final O(k) reduction, replicating the reference numerics ----
    h_topk = h[topk_idx]
    log_w = np.log(topk_w + np.float32(EPS)) + np.log(h_topk + np.float32(EPS))
    m = log_w.max(axis=-1, keepdims=True)
    ew = np.exp(log_w - m)
    weights = (ew / ew.sum(axis=-1, keepdims=True)).astype(np.float32)

    V_sel = V[topk_idx]  # [Q, 32, DV]
    e_sel = e[topk_idx]  # [Q, 32, DE]
    r_V = np.einsum('qk,qkv->qv', weights, V_sel).astype(np.float32)
    r_E = np.einsum('qk,qke->qe', weights, e_sel).astype(np.float32)

    return (
        r_V.reshape(B, T, DV),
        r_E.reshape(B, T, DE),
        weights.reshape(B, T, top_k),
        topk_idx.reshape(B, T, top_k).astype(np.int32),
    )


# revision 20
# speedup vs baseline: 1.0259x; 1.0259x over previous
"""Distributed MemoryCenters read kernel for 8 Trainium2 NeuronCores.

Strategy (sharded-kNN per the distributed top-k pattern):
  - Shard the center table K along n_centers across the 8 cores
    (12500 centers each). Queries are replicated.
  - Each core computes sim = q @ K_shard^T on the PE. To get fp32-grade
    precision at full PE rate, operands are split hi/lo (hi = fp16 with
    subnormals flushed on host, lo = bf16 residual) and accumulated as
    qh*Kh + qh*Kl + ql*Kh in one PSUM group (measured max err 1.9e-7,
    same as fp32).
  - Each core extracts top-8 candidates (values + in-chunk indices) per
    2048-wide chunk of its shard with the DVE max8 / find_index8 ops.
  - The host merges the 8 * 56 = 448 candidates per query, takes the
    global top-32 by RBF weight (reproducing the reference's ordering and
    tie-breaking), and performs the cheap O(k) softmax / gather reduction.

Exactness: top-8 per 2048-chunk covers the global top-32 as long as no
chunk holds more than 8 of the top-32 (actual maximum on this data is 5;
test.py's saturation check proves this per-run).
"""

import numpy as np

SIGMA_READ = 0.5
EPS = 1e-8

B, T, D = 2, 512, 128
N, DV, DE = 100000, 256, 4
NCORES = 8
NSHARD = N // NCORES  # 12500
Q = B * T  # 1024
QTILES = Q // 128  # 8

CHUNK = 2048


def _mk_chunks(widths):
    assert sum(widths) == NSHARD
    out, off = [], 0
    for w in widths:
        out.append((off, w))
        off += w
    return out


# Tile 0 uses small leading chunks so the DVE starts as soon as the first
# matmul pieces land (pipeline fill); later tiles use coarse chunks to
# minimize per-instruction overhead on the saturated DVE.
CHUNKS0 = _mk_chunks([512, 512, 1024, 2048, 2048, 2048, 2048, 2260])
CHUNKSR = _mk_chunks([2048, 2048, 2048, 2048, 2048, 2260])
CAND0 = len(CHUNKS0) * 8  # 64
CANDR = len(CHUNKSR) * 8  # 48
CAND = CAND0  # DRAM output width (tiles 1+ use the first CANDR slots)
CHUNKS = CHUNKSR  # coarse list (used by saturation checks)

F16_MIN_NORMAL = 6.2e-05  # flush-to-zero threshold for the hi fp16 part

_CACHE = {}


def _build_bass():
    """Build + compile the per-core Bass program (identical on all cores)."""
    from contextlib import ExitStack

    import concourse.bacc as bacc
    import concourse.mybir as mybir
    import concourse.tile as tile

    f32 = mybir.dt.float32
    f16 = mybir.dt.float16
    bf16 = mybir.dt.bfloat16
    u32 = mybir.dt.uint32

    nc = bacc.Bacc("TRN2", target_bir_lowering=False, debug=False,
                   enable_asserts=False, num_devices=NCORES)

    qh = nc.dram_tensor("qh", [128, Q], f16, kind="ExternalInput").ap()
    ql = nc.dram_tensor("ql", [128, Q], bf16, kind="ExternalInput").ap()
    kh = nc.dram_tensor("kh", [128, NSHARD], f16, kind="ExternalInput").ap()
    kl = nc.dram_tensor("kl", [128, NSHARD], bf16, kind="ExternalInput").ap()
    outv = nc.dram_tensor("outv", [QTILES, 128, CAND], f32, kind="ExternalOutput").ap()
    outi = nc.dram_tensor("outi", [QTILES, 128, CAND], u32, kind="ExternalOutput").ap()

    with tile.TileContext(nc) as tc, ExitStack() as ctx:
        k_pool = ctx.enter_context(tc.tile_pool(name="k", bufs=1))
        q_pool = ctx.enter_context(tc.tile_pool(name="q", bufs=1))
        # 2-bank PSUM pieces, 4 in flight: lets the PE run ahead of the
        # ACT drain so it can ramp to its top p-state
        psum_pool = ctx.enter_context(tc.tile_pool(name="ps", bufs=4, space="PSUM"))
        # deep SBUF runway so ACT (and transitively PE) is not paced by DVE
        chunk_pool = ctx.enter_context(tc.tile_pool(name="ch", bufs=7))
        out_pool = ctx.enter_context(tc.tile_pool(name="out", bufs=2))

        # queries first (the PE's ldweights gate on them), on both HWDGE rings
        qh_sb = q_pool.tile([128, Q], f16, tag="qh")
        ql_sb = q_pool.tile([128, Q], bf16, tag="ql")
        nc.sync.dma_start(out=qh_sb[:], in_=qh[:])
        nc.scalar.dma_start(out=ql_sb[:], in_=ql[:])

        # K in chunk-aligned pieces as separate tiles so early matmuls only
        # gate on the piece they read; kh on the SP ring, kl on the ACT ring
        piece_bounds = [0, 512, 2048, 6144, 10240, NSHARD]
        pieces = [(piece_bounds[i], piece_bounds[i + 1] - piece_bounds[i])
                  for i in range(len(piece_bounds) - 1)]
        kh_sbs, kl_sbs = [], []
        for pi, (s, w) in enumerate(pieces):
            kh_t = k_pool.tile([128, w], f16, tag=f"kh{pi}")
            kl_t = k_pool.tile([128, w], bf16, tag=f"kl{pi}")
            nc.sync.dma_start(out=kh_t[:], in_=kh[:, s:s + w])
            nc.scalar.dma_start(out=kl_t[:], in_=kl[:, s:s + w])
            kh_sbs.append(kh_t)
            kl_sbs.append(kl_t)

        def k_slice(tiles, off, width):
            for (s, w), t in zip(pieces, tiles):
                if s <= off and off + width <= s + w:
                    return t[:, off - s:off - s + width]
            raise AssertionError((off, width))

        for t in range(QTILES):
            chunks = CHUNKS0 if t == 0 else CHUNKSR
            ncand = len(chunks) * 8
            ov = out_pool.tile([128, CAND], f32, tag="ov")
            oi = out_pool.tile([128, CAND], u32, tag="oi")
            lh = qh_sb[:, t * 128:(t + 1) * 128]
            ll = ql_sb[:, t * 128:(t + 1) * 128]
            for ci, (off, w) in enumerate(chunks):
                sb = chunk_pool.tile([128, 2260], f32, tag="ch")
                # PSUM pieces of 1024 (2 banks); ACT drains each piece into
                # its slice of the chunk's SBUF buffer
                for p0 in range(0, w, 1024):
                    pw = min(1024, w - p0)
                    ps = psum_pool.tile([128, 1024], f32, tag="ps")
                    for j in range(0, pw, 512):
                        ww = min(512, pw - j)
                        rh = k_slice(kh_sbs, off + p0 + j, ww)
                        rl = k_slice(kl_sbs, off + p0 + j, ww)
                        po = ps[:, j:j + ww]
                        nc.tensor.matmul(po, lhsT=lh, rhs=rh, start=True, stop=False)
                        nc.tensor.matmul(po, lhsT=lh, rhs=rl, start=False, stop=False)
                        nc.tensor.matmul(po, lhsT=ll, rhs=rh, start=False, stop=True)
                    nc.scalar.activation(sb[:, p0:p0 + pw], ps[:, :pw],
                                         mybir.ActivationFunctionType.Copy)
                nc.vector.max(ov[:, ci * 8:(ci + 1) * 8], sb[:, :w])
                nc.vector.max_index(oi[:, ci * 8:(ci + 1) * 8],
                                    ov[:, ci * 8:(ci + 1) * 8], sb[:, :w])
            nc.sync.dma_start(out=outv[t, :, :ncand], in_=ov[:, :ncand])
            nc.sync.dma_start(out=outi[t, :, :ncand], in_=oi[:, :ncand])

    nc.compile()
    return nc


def _get_compiled():
    if "nc" not in _CACHE:
        _CACHE["nc"] = _build_bass()
    return _CACHE["nc"]


def _split_hi_lo(x):
    """x (f32) -> (hi fp16 with subnormals flushed, lo bf16), hi+lo ~ x."""
    import ml_dtypes
    xh = x.astype(np.float16)
    xh = np.where(np.abs(x) < F16_MIN_NORMAL, np.float16(0), xh)
    xl = (x - xh.astype(np.float32)).astype(ml_dtypes.bfloat16)
    return xh, xl


def build_in_maps(queries, K):
    qT = np.ascontiguousarray(queries.reshape(Q, D).T)  # [128, 1024]
    qh, ql = _split_hi_lo(qT)
    in_maps = []
    for c in range(NCORES):
        sh = np.ascontiguousarray(K[c * NSHARD:(c + 1) * NSHARD].T)  # [128, 12500]
        kh, kl = _split_hi_lo(sh)
        in_maps.append({"qh": qh, "ql": ql, "kh": kh, "kl": kl})
    return in_maps


def kernel(queries, K, V, h, e, top_k):
    top_k = int(top_k)
    assert top_k == 32, top_k
    queries = np.asarray(queries, dtype=np.float32)
    K = np.asarray(K, dtype=np.float32)
    V = np.asarray(V, dtype=np.float32)
    h = np.asarray(h, dtype=np.float32)
    e = np.asarray(e, dtype=np.float32)

    # ---- shard + run on 8 NeuronCores ----
    in_maps = build_in_maps(queries, K)

    from concourse.bass_utils import run_bass_kernel_spmd

    nc = _get_compiled()
    w0 = np.repeat(np.array([w for _, w in CHUNKS0], dtype=np.int64), 8)
    wR = np.repeat(np.array([w for _, w in CHUNKSR], dtype=np.int64), 8)

    def _outputs_sane(res):
        # sims of unit vectors lie in [-1, 1]; indices are in-chunk offsets
        for c in range(NCORES):
            v = res[c]["outv"]
            if not np.isfinite(v).all() or np.abs(v).max() > 1.001:
                return False
            i_ = res[c]["outi"].astype(np.int64)
            if (i_[0, :, :CAND0] >= w0).any() or (i_[1:, :, :CANDR] >= wR).any():
                return False
        return True

    res = run_bass_kernel_spmd(nc, in_maps, list(range(NCORES))).results
    if not _outputs_sane(res):  # transient device glitch: retry once
        res = run_bass_kernel_spmd(nc, in_maps, list(range(NCORES))).results

    # ---- unshard: merge the per-core candidates per query ----
    vals = np.stack([res[c]["outv"] for c in range(NCORES)])  # [8, QTILES, 128, CAND]
    idxs = np.stack([res[c]["outi"] for c in range(NCORES)]).astype(np.int64)
    off0 = np.repeat(np.array([off for off, _ in CHUNKS0], dtype=np.int64), 8)
    offR = np.repeat(np.array([off for off, _ in CHUNKSR], dtype=np.int64), 8)
    gidx = idxs
    gidx[:, 0] += off0[None, None, :]
    gidx[:, 1:, :, :CANDR] += offR[None, None, None, :]
    # tiles 1+ only populate the first CANDR slots; neutralize the rest
    vals[:, 1:, :, CANDR:] = -4.0
    gidx[:, 1:, :, CANDR:] = 0
    gidx += (np.arange(NCORES, dtype=np.int64) * NSHARD)[:, None, None, None]

    v = vals.transpose(1, 2, 0, 3).reshape(Q, NCORES * CAND)
    gi = gidx.transpose(1, 2, 0, 3).reshape(Q, NCORES * CAND)

    # RBF weights, computed exactly as the reference does (f32 throughout)
    dist_sq = np.float32(2.0) - np.float32(2.0) * v
    rbf = np.exp(-dist_sq / np.float32(2.0 * SIGMA_READ ** 2)).astype(np.float32)

    # global top-32 by rbf, ties broken by lower center index (lax.top_k order)
    order = np.lexsort((gi, -rbf.astype(np.float64)), axis=1)[:, :top_k]
    topk_idx = np.take_along_axis(gi, order, axis=1)  # [Q, 32]
    topk_w = np.take_along_axis(rbf, order, axis=1)  # [Q, 32]

    # ---- final O(k) reduction, replicating the reference numerics ----
    h_topk = h[topk_idx]
    log_w = np.log(topk_w + np.float32(EPS)) + np.log(h_topk + np.float32(EPS))
    m = log_w.max(axis=-1, keepdims=True)
    ew = np.exp(log_w - m)
    weights = (ew / ew.sum(axis=-1, keepdims=True)).astype(np.float32)

    V_sel = V[topk_idx]  # [Q, 32, DV]
    e_sel = e[topk_idx]  # [Q, 32, DE]
    r_V = np.einsum('qk,qkv->qv', weights, V_sel).astype(np.float32)
    r_E = np.einsum('qk,qke->qe', weights, e_sel).astype(np.float32)

    return (
        r_V.reshape(B, T, DV),
        r_E.reshape(B, T, DE),
        weights.reshape(B, T, top_k),
        topk_idx.reshape(B, T, top_k).astype(np.int32),
    )


# revision 23
# speedup vs baseline: 1.0398x; 1.0136x over previous
"""Distributed MemoryCenters read kernel for 8 Trainium2 NeuronCores.

Strategy (sharded-kNN per the distributed top-k pattern):
  - Shard the center table K along n_centers across the 8 cores
    (12500 centers each). Queries are replicated.
  - Each core computes sim = q @ K_shard^T on the PE. To get fp32-grade
    precision at full PE rate, operands are split hi/lo (hi = fp16 with
    subnormals flushed on host, lo = bf16 residual) and accumulated as
    qh*Kh + qh*Kl + ql*Kh in one PSUM group (measured max err 1.9e-7,
    same as fp32).
  - Each core extracts top-8 candidates (values + in-chunk indices) per
    2048-wide chunk of its shard with the DVE max8 / find_index8 ops.
  - The host merges the 8 * 56 = 448 candidates per query, takes the
    global top-32 by RBF weight (reproducing the reference's ordering and
    tie-breaking), and performs the cheap O(k) softmax / gather reduction.

Exactness: top-8 per 2048-chunk covers the global top-32 as long as no
chunk holds more than 8 of the top-32 (actual maximum on this data is 5;
test.py's saturation check proves this per-run).
"""

import numpy as np

SIGMA_READ = 0.5
EPS = 1e-8

B, T, D = 2, 512, 128
N, DV, DE = 100000, 256, 4
NCORES = 8
NSHARD = N // NCORES  # 12500
Q = B * T  # 1024
QTILES = Q // 128  # 8

CHUNK = 2048


def _mk_chunks(widths):
    assert sum(widths) == NSHARD
    out, off = [], 0
    for w in widths:
        out.append((off, w))
        off += w
    return out


# Tile 0 uses small leading chunks so the DVE starts as soon as the first
# matmul pieces land (pipeline fill); later tiles use coarse chunks to
# minimize per-instruction overhead on the saturated DVE.
CHUNKS0 = _mk_chunks([512, 512, 1476, 2500, 2500, 2500, 2500])
CHUNKSR = _mk_chunks([2500, 2500, 2500, 2500, 2500])
CAND0 = len(CHUNKS0) * 8  # 64
CANDR = len(CHUNKSR) * 8  # 48
CAND = CAND0  # DRAM output width (tiles 1+ use the first CANDR slots)
CHUNKS = CHUNKSR  # coarse list (used by saturation checks)

F16_MIN_NORMAL = 6.2e-05  # flush-to-zero threshold for the hi fp16 part

_CACHE = {}


def _build_bass():
    """Build + compile the per-core Bass program (identical on all cores)."""
    from contextlib import ExitStack

    import concourse.bacc as bacc
    import concourse.mybir as mybir
    import concourse.tile as tile

    f32 = mybir.dt.float32
    f16 = mybir.dt.float16
    bf16 = mybir.dt.bfloat16
    u32 = mybir.dt.uint32

    nc = bacc.Bacc("TRN2", target_bir_lowering=False, debug=False,
                   enable_asserts=False, num_devices=NCORES)

    qh = nc.dram_tensor("qh", [128, Q], f16, kind="ExternalInput").ap()
    ql = nc.dram_tensor("ql", [128, Q], bf16, kind="ExternalInput").ap()
    kh = nc.dram_tensor("kh", [128, NSHARD], f16, kind="ExternalInput").ap()
    kl = nc.dram_tensor("kl", [128, NSHARD], bf16, kind="ExternalInput").ap()
    outv = nc.dram_tensor("outv", [QTILES, 128, CAND], f32, kind="ExternalOutput").ap()
    outi = nc.dram_tensor("outi", [QTILES, 128, CAND], u32, kind="ExternalOutput").ap()

    with tile.TileContext(nc) as tc, ExitStack() as ctx:
        k_pool = ctx.enter_context(tc.tile_pool(name="k", bufs=1))
        q_pool = ctx.enter_context(tc.tile_pool(name="q", bufs=1))
        # 2-bank PSUM pieces, 4 in flight: lets the PE run ahead of the
        # ACT drain so it can ramp to its top p-state
        psum_pool = ctx.enter_context(tc.tile_pool(name="ps", bufs=4, space="PSUM"))
        # deep SBUF runway so ACT (and transitively PE) is not paced by DVE
        chunk_pool = ctx.enter_context(tc.tile_pool(name="ch", bufs=7))
        out_pool = ctx.enter_context(tc.tile_pool(name="out", bufs=2))

        # queries first (the PE's ldweights gate on them), on both HWDGE rings
        qh_sb = q_pool.tile([128, Q], f16, tag="qh")
        ql_sb = q_pool.tile([128, Q], bf16, tag="ql")
        nc.sync.dma_start(out=qh_sb[:], in_=qh[:])
        nc.scalar.dma_start(out=ql_sb[:], in_=ql[:])

        # K in chunk-aligned pieces as separate tiles so early matmuls only
        # gate on the piece they read; kh on the SP ring, kl on the ACT ring
        piece_bounds = [0, 512, 2500, 5000, 7500, 10000, NSHARD]
        pieces = [(piece_bounds[i], piece_bounds[i + 1] - piece_bounds[i])
                  for i in range(len(piece_bounds) - 1)]
        kh_sbs, kl_sbs = [], []
        for pi, (s, w) in enumerate(pieces):
            kh_t = k_pool.tile([128, w], f16, tag=f"kh{pi}")
            kl_t = k_pool.tile([128, w], bf16, tag=f"kl{pi}")
            nc.sync.dma_start(out=kh_t[:], in_=kh[:, s:s + w])
            nc.scalar.dma_start(out=kl_t[:], in_=kl[:, s:s + w])
            kh_sbs.append(kh_t)
            kl_sbs.append(kl_t)

        def k_slice(tiles, off, width):
            for (s, w), t in zip(pieces, tiles):
                if s <= off and off + width <= s + w:
                    return t[:, off - s:off - s + width]
            raise AssertionError((off, width))

        for t in range(QTILES):
            chunks = CHUNKS0 if t == 0 else CHUNKSR
            ncand = len(chunks) * 8
            ov = out_pool.tile([128, CAND], f32, tag="ov")
            oi = out_pool.tile([128, CAND], u32, tag="oi")
            lh = qh_sb[:, t * 128:(t + 1) * 128]
            ll = ql_sb[:, t * 128:(t + 1) * 128]
            for ci, (off, w) in enumerate(chunks):
                sb = chunk_pool.tile([128, 2500], f32, tag="ch")
                # PSUM pieces of 1024 (2 banks); ACT drains each piece into
                # its slice of the chunk's SBUF buffer
                for p0 in range(0, w, 1024):
                    pw = min(1024, w - p0)
                    ps = psum_pool.tile([128, 1024], f32, tag="ps")
                    for j in range(0, pw, 512):
                        ww = min(512, pw - j)
                        rh = k_slice(kh_sbs, off + p0 + j, ww)
                        rl = k_slice(kl_sbs, off + p0 + j, ww)
                        po = ps[:, j:j + ww]
                        nc.tensor.matmul(po, lhsT=lh, rhs=rh, start=True, stop=False)
                        nc.tensor.matmul(po, lhsT=lh, rhs=rl, start=False, stop=False)
                        nc.tensor.matmul(po, lhsT=ll, rhs=rh, start=False, stop=True)
                    nc.scalar.activation(sb[:, p0:p0 + pw], ps[:, :pw],
                                         mybir.ActivationFunctionType.Copy)
                nc.vector.max(ov[:, ci * 8:(ci + 1) * 8], sb[:, :w])
                nc.vector.max_index(oi[:, ci * 8:(ci + 1) * 8],
                                    ov[:, ci * 8:(ci + 1) * 8], sb[:, :w])
            nc.sync.dma_start(out=outv[t, :, :ncand], in_=ov[:, :ncand])
            nc.sync.dma_start(out=outi[t, :, :ncand], in_=oi[:, :ncand])

    nc.compile()
    return nc


def _get_compiled():
    if "nc" not in _CACHE:
        _CACHE["nc"] = _build_bass()
    return _CACHE["nc"]


def _split_hi_lo(x):
    """x (f32) -> (hi fp16 with subnormals flushed, lo bf16), hi+lo ~ x."""
    import ml_dtypes
    xh = x.astype(np.float16)
    xh = np.where(np.abs(x) < F16_MIN_NORMAL, np.float16(0), xh)
    xl = (x - xh.astype(np.float32)).astype(ml_dtypes.bfloat16)
    return xh, xl


def build_in_maps(queries, K):
    qT = np.ascontiguousarray(queries.reshape(Q, D).T)  # [128, 1024]
    qh, ql = _split_hi_lo(qT)
    in_maps = []
    for c in range(NCORES):
        sh = np.ascontiguousarray(K[c * NSHARD:(c + 1) * NSHARD].T)  # [128, 12500]
        kh, kl = _split_hi_lo(sh)
        in_maps.append({"qh": qh, "ql": ql, "kh": kh, "kl": kl})
    return in_maps


def kernel(queries, K, V, h, e, top_k):
    top_k = int(top_k)
    assert top_k == 32, top_k
    queries = np.asarray(queries, dtype=np.float32)
    K = np.asarray(K, dtype=np.float32)
    V = np.asarray(V, dtype=np.float32)
    h = np.asarray(h, dtype=np.float32)
    e = np.asarray(e, dtype=np.float32)

    # ---- shard + run on 8 NeuronCores ----
    in_maps = build_in_maps(queries, K)

    from concourse.bass_utils import run_bass_kernel_spmd

    nc = _get_compiled()
    w0 = np.repeat(np.array([w for _, w in CHUNKS0], dtype=np.int64), 8)
    wR = np.repeat(np.array([w for _, w in CHUNKSR], dtype=np.int64), 8)

    def _outputs_sane(res):
        # sims of unit vectors lie in [-1, 1]; indices are in-chunk offsets
        for c in range(NCORES):
            v = res[c]["outv"]
            if not np.isfinite(v).all() or np.abs(v).max() > 1.001:
                return False
            i_ = res[c]["outi"].astype(np.int64)
            if (i_[0, :, :CAND0] >= w0).any() or (i_[1:, :, :CANDR] >= wR).any():
                return False
        return True

    res = run_bass_kernel_spmd(nc, in_maps, list(range(NCORES))).results
    if not _outputs_sane(res):  # transient device glitch: retry once
        res = run_bass_kernel_spmd(nc, in_maps, list(range(NCORES))).results

    # ---- unshard: merge the per-core candidates per query ----
    vals = np.stack([res[c]["outv"] for c in range(NCORES)])  # [8, QTILES, 128, CAND]
    idxs = np.stack([res[c]["outi"] for c in range(NCORES)]).astype(np.int64)
    off0 = np.repeat(np.array([off for off, _ in CHUNKS0], dtype=np.int64), 8)
    offR = np.repeat(np.array([off for off, _ in CHUNKSR], dtype=np.int64), 8)
    gidx = idxs
    gidx[:, 0] += off0[None, None, :]
    gidx[:, 1:, :, :CANDR] += offR[None, None, None, :]
    # tiles 1+ only populate the first CANDR slots; neutralize the rest
    vals[:, 1:, :, CANDR:] = -4.0
    gidx[:, 1:, :, CANDR:] = 0
    gidx += (np.arange(NCORES, dtype=np.int64) * NSHARD)[:, None, None, None]

    v = vals.transpose(1, 2, 0, 3).reshape(Q, NCORES * CAND)
    gi = gidx.transpose(1, 2, 0, 3).reshape(Q, NCORES * CAND)

    # RBF weights, computed exactly as the reference does (f32 throughout)
    dist_sq = np.float32(2.0) - np.float32(2.0) * v
    rbf = np.exp(-dist_sq / np.float32(2.0 * SIGMA_READ ** 2)).astype(np.float32)

    # global top-32 by rbf, ties broken by lower center index (lax.top_k order)
    order = np.lexsort((gi, -rbf.astype(np.float64)), axis=1)[:, :top_k]
    topk_idx = np.take_along_axis(gi, order, axis=1)  # [Q, 32]
    topk_w = np.take_along_axis(rbf, order, axis=1)  # [Q, 32]

    # ---- final O(k) reduction, replicating the reference numerics ----
    h_topk = h[topk_idx]
    log_w = np.log(topk_w + np.float32(EPS)) + np.log(h_topk + np.float32(EPS))
    m = log_w.max(axis=-1, keepdims=True)
    ew = np.exp(log_w - m)
    weights = (ew / ew.sum(axis=-1, keepdims=True)).astype(np.float32)

    V_sel = V[topk_idx]  # [Q, 32, DV]
    e_sel = e[topk_idx]  # [Q, 32, DE]
    r_V = np.einsum('qk,qkv->qv', weights, V_sel).astype(np.float32)
    r_E = np.einsum('qk,qke->qe', weights, e_sel).astype(np.float32)

    return (
        r_V.reshape(B, T, DV),
        r_E.reshape(B, T, DE),
        weights.reshape(B, T, top_k),
        topk_idx.reshape(B, T, top_k).astype(np.int32),
    )
